# revision 13
# baseline (speedup 1.0000x reference)
"""Trainium2 Bass kernel for nn_KnowledgeCriterion (ComplEx-style loss).

Full (unsharded) inputs:
  tri_feat_org: (256, 128, 1536) f32
  alpha:        (256, 64, 128)   f32
  mask:         (256, 64)        f32
Output: scalar f32 loss.

Strategy: data-parallel over batch on 8 NeuronCores (32 batches/core).
Each core computes three partial scalars (softplus-sum, regul-dot, mask-sum);
host combines:  loss = sp/numtrue + 0.01 * regul_dot/(B*S*R*D).

Per-batch on-chip pipeline (feature tile X = (R=128 part, F=1536 free)):
  s0[r] = sum_d [ r_re*(h_re*t_re + h_im*t_im) + r_im*(h_re*t_im - h_im*t_re) ]
     - DVE: A=h_re*t_re, B=h_im*t_im, then tensor_tensor_reduce chain with r_re
     - GPSIMD: Dp=h_re*t_im, Ep=h_im*t_re, scalar_tensor_tensor accums with r_im
  regul_dot += sum_r a2s[r] * sum_f X[r,f]^2
     - ACT Square -> X2; PE matmul (stationary=a2s col) accumulating in PSUM
  score = -(a^3)*s0, a=(alpha-0.1)*mask   (alpha transposed to (R,S) via PE)
  softplus(score) = (score+|score|)/2 + ln(1+exp(-|score|))
     - DVE tensor_scalar accum -> sum(score); GPSIMD STT max -> |score| + accum
     - ACT Exp(scale=-1), Ln(bias=1) + accum
"""
import numpy as np

B, S, R, F = 256, 64, 128, 1536
D = F // 6
N_CORES = 8
B_LOC = B // N_CORES

_CACHE = {}


def _build_nc():
    import concourse.bacc as bacc
    import concourse.tile as tile
    import concourse.masks as masks
    from concourse import mybir

    F32 = mybir.dt.float32
    BF16 = mybir.dt.bfloat16
    ALU = mybir.AluOpType
    ACTF = mybir.ActivationFunctionType

    nc = bacc.Bacc("TRN2", target_bir_lowering=False, debug=False)
    feat = nc.dram_tensor("feat", [B_LOC, R, F], F32, kind="ExternalInput")
    alph = nc.dram_tensor("alpha", [B_LOC, S, R], F32, kind="ExternalInput")
    msk = nc.dram_tensor("mask", [B_LOC, S, 1], F32, kind="ExternalInput")
    outp = nc.dram_tensor("partials", [1, 4], F32, kind="ExternalOutput")

    with tile.TileContext(nc) as tc:
        with (
            tc.tile_pool(name="const", bufs=1) as constp,
            tc.tile_pool(name="xf", bufs=4) as xf,
            tc.tile_pool(name="x2", bufs=2) as x2p,
            tc.tile_pool(name="prod", bufs=2) as prod,
            tc.tile_pool(name="alp", bufs=2) as alp,
            tc.tile_pool(name="sco", bufs=2) as sco,
            tc.tile_pool(name="cols", bufs=2) as colsp,
            tc.tile_pool(name="accum", bufs=1) as accp,
            tc.tile_pool(name="fin", bufs=1) as finp,
            tc.tile_pool(name="pst", bufs=2, space="PSUM") as pst,
            tc.tile_pool(name="psr", bufs=1, space="PSUM") as psr,
            tc.tile_pool(name="psf", bufs=1, space="PSUM") as psf,
        ):
            ident = constp.tile([128, 128], F32)
            masks.make_identity(nc, ident[:])
            ones = constp.tile([128, 1], F32)
            nc.gpsimd.memset(ones[:], 1.0)

            # accumulation buffers: one column per batch
            lsums = accp.tile([128, B_LOC], F32)
            xsums = accp.tile([128, B_LOC], F32)
            absums = accp.tile([128, B_LOC], F32)
            mask_cols = accp.tile([S, B_LOC], F32)

            # persistent PSUM accumulators for regul (3 chunks of 512)
            rg_ps = [psr.tile([1, 512], F32, name=f"rg_ps{k}", tag=f"rg{k}")
                     for k in range(3)]

            for b in range(B_LOC):
                # ---- loads ----
                X = xf.tile([R, F], F32)
                nc.sync.dma_start(X[:], feat.ap()[b])
                alt = alp.tile([S, R], F32)
                nc.sync.dma_start(alt[:], alph.ap()[b])
                nc.sync.dma_start(mask_cols[:, b:b + 1], msk.ap()[b])

                h_re = X[:, 0 * D:1 * D]
                h_im = X[:, 1 * D:2 * D]
                r_re = X[:, 2 * D:3 * D]
                r_im = X[:, 3 * D:4 * D]
                t_re = X[:, 4 * D:5 * D]
                t_im = X[:, 5 * D:6 * D]

                # ---- alpha side ----
                am = alp.tile([S, R], F32, tag="am")
                nc.vector.tensor_scalar(
                    out=am[:], in0=alt[:], scalar1=0.1, scalar2=mask_cols[:, b:b + 1],
                    op0=ALU.subtract, op1=ALU.mult)
                amT_ps = pst.tile([R, S], F32, tag="amT_ps")
                nc.tensor.transpose(amT_ps[:], am[:], ident[:S, :S])
                amT = alp.tile([R, S], F32, tag="amT")
                nc.scalar.copy(amT[:], amT_ps[:])

                a2T = sco.tile([R, S], F32, tag="a2T")
                a2s = colsp.tile([R, 1], F32, tag="a2s")
                nc.vector.scalar_tensor_tensor(
                    out=a2T[:], in0=amT[:], scalar=1.0, in1=amT[:],
                    op0=ALU.mult, op1=ALU.mult, accum_out=a2s[:])
                a3T = sco.tile([R, S], F32, tag="a3T")
                nc.vector.tensor_tensor(out=a3T[:], in0=a2T[:], in1=amT[:], op=ALU.mult)

                # ---- feature side: squares for regul (bf16 for full-rate PE;
                # regul is a 1e-4-scale term of the output so bf16 is ample) ----
                a2sb = colsp.tile([R, 1], BF16, tag="a2sb")
                nc.vector.tensor_copy(a2sb[:], a2s[:])
                X2 = x2p.tile([R, F], BF16)
                nc.scalar.activation(out=X2[:], in_=X[:], func=ACTF.Square)
                for k in range(3):
                    nc.tensor.matmul(
                        rg_ps[k][:], a2sb[:], X2[:, k * 512:(k + 1) * 512],
                        start=(b == 0), stop=(b == B_LOC - 1))

                # ---- feature side: s0 ----
                A = prod.tile([R, D], F32, tag="A")
                nc.vector.tensor_tensor(out=A[:], in0=h_re, in1=t_re, op=ALU.mult)
                Bt = prod.tile([R, D], F32, tag="B")
                nc.vector.tensor_tensor(out=Bt[:], in0=h_im, in1=t_im, op=ALU.mult)
                Dp = prod.tile([R, D], F32, tag="Dp")
                nc.gpsimd.tensor_tensor(out=Dp[:], in0=h_re, in1=t_im, op=ALU.mult)
                Ep = prod.tile([R, D], F32, tag="Ep")
                nc.gpsimd.tensor_tensor(out=Ep[:], in0=h_im, in1=t_re, op=ALU.mult)

                # negs0 = -s0 via 4 STT multiply-accumulates (signs folded into
                # the per-op scalar), partials land in 4 columns of one tile:
                #   -sum(A*r_re) - sum(B*r_re) - sum(Dp*r_im) + sum(Ep*r_im)
                parts4 = colsp.tile([R, 4], F32, tag="parts4")
                jA = prod.tile([R, D], F32, tag="jA")
                nc.vector.scalar_tensor_tensor(
                    out=jA[:], in0=A[:], scalar=-1.0, in1=r_re,
                    op0=ALU.mult, op1=ALU.mult, accum_out=parts4[:, 0:1])
                jB = prod.tile([R, D], F32, tag="jB")
                nc.vector.scalar_tensor_tensor(
                    out=jB[:], in0=Bt[:], scalar=-1.0, in1=r_re,
                    op0=ALU.mult, op1=ALU.mult, accum_out=parts4[:, 1:2])
                jD = prod.tile([R, D], F32, tag="jD")
                nc.vector.scalar_tensor_tensor(
                    out=jD[:], in0=Dp[:], scalar=-1.0, in1=r_im,
                    op0=ALU.mult, op1=ALU.mult, accum_out=parts4[:, 2:3])
                jE = prod.tile([R, D], F32, tag="jE")
                nc.vector.scalar_tensor_tensor(
                    out=jE[:], in0=Ep[:], scalar=1.0, in1=r_im,
                    op0=ALU.mult, op1=ALU.mult, accum_out=parts4[:, 3:4])
                negs0 = colsp.tile([R, 1], F32, tag="negs0")
                nc.vector.tensor_reduce(
                    out=negs0[:], in_=parts4[:], axis=mybir.AxisListType.X,
                    op=ALU.add)

                # ---- score & softplus ----
                scoreT = sco.tile([R, S], F32, tag="scoreT")
                nc.vector.tensor_scalar(
                    out=scoreT[:], in0=a3T[:], scalar1=negs0[:], scalar2=0.0,
                    op0=ALU.mult, op1=ALU.add, accum_out=xsums[:, b:b + 1])
                absT = sco.tile([R, S], F32, tag="absT")
                nc.scalar.activation(
                    out=absT[:], in_=scoreT[:], func=ACTF.Abs,
                    accum_out=absums[:, b:b + 1])
                expT = sco.tile([R, S], F32, tag="expT")
                nc.scalar.activation(out=expT[:], in_=absT[:], func=ACTF.Exp, scale=-1.0)
                lnT = sco.tile([R, S], F32, tag="lnT")
                nc.scalar.activation(
                    out=lnT[:], in_=expT[:], func=ACTF.Ln, bias=1.0,
                    accum_out=lsums[:, b:b + 1])

            # ---- endgame ----
            ALUm = ALU
            v = finp.tile([128, B_LOC], F32)
            nc.vector.tensor_tensor(out=v[:], in0=xsums[:], in1=absums[:], op=ALUm.add)
            v2 = finp.tile([128, B_LOC], F32)
            nc.vector.scalar_tensor_tensor(
                out=v2[:], in0=v[:], scalar=0.5, in1=lsums[:],
                op0=ALUm.mult, op1=ALUm.add)
            spv = finp.tile([128, 1], F32)
            nc.vector.tensor_reduce(
                out=spv[:], in_=v2[:], axis=mybir.AxisListType.X, op=ALUm.add)

            mred = finp.tile([S, 1], F32)
            nc.vector.tensor_reduce(
                out=mred[:], in_=mask_cols[:], axis=mybir.AxisListType.X, op=ALUm.add)

            rgsb = finp.tile([1, F], F32)
            for k in range(3):
                nc.scalar.copy(rgsb[:, k * 512:(k + 1) * 512], rg_ps[k][:])
            rgs = finp.tile([1, 1], F32)
            nc.vector.tensor_reduce(
                out=rgs[:], in_=rgsb[:], axis=mybir.AxisListType.X, op=ALUm.add)

            fin_ps = psf.tile([1, 4], F32)
            nc.tensor.matmul(fin_ps[:, 0:1], spv[:], ones[:], start=True, stop=True)
            nc.tensor.matmul(fin_ps[:, 2:3], mred[:], ones[:S, :], start=True, stop=True)

            out_sb = finp.tile([1, 4], F32)
            nc.scalar.copy(out_sb[:, 0:1], fin_ps[:, 0:1])
            nc.scalar.copy(out_sb[:, 1:2], rgs[:])
            nc.scalar.copy(out_sb[:, 2:3], fin_ps[:, 2:3])
            nc.gpsimd.memset(out_sb[:, 3:4], 0.0)
            nc.sync.dma_start(outp.ap(), out_sb[:])

    nc.compile()

    # Collapse the act-table loads: every activation used (square, abs, exp,
    # ln, copy, identity) lives in set 6 = natural_log_exp_and_others, but the
    # greedy inserter alternates sets 0/5 (one reload per iteration, ~1.3us
    # each). Pin the first load to set 6 and drop the rest (they carry no
    # sync info).
    first = True
    for bb in nc.m.functions[0].blocks:
        keep = []
        for inst in bb.instructions:
            if isinstance(inst, mybir.InstLoadActFuncSet):
                si = inst.sync_info
                assert not (si and (si.on_wait or si.on_update))
                if first:
                    inst.act_func_set_id = 6
                    first = False
                    keep.append(inst)
            else:
                keep.append(inst)
        if len(keep) != len(bb.instructions):
            il = bb.instructions
            il[:] = keep
    return nc


def _get_nc():
    if "nc" not in _CACHE:
        _CACHE["nc"] = _build_nc()
    return _CACHE["nc"]


def _get_runner():
    """Persistent jitted 8-core runner (mirrors bass2jax.run_bass_via_pjrt)."""
    if "runner" in _CACHE:
        return _CACHE["runner"]
    import jax
    from jax.sharding import Mesh, PartitionSpec
    from jax.experimental.shard_map import shard_map
    import concourse.mybir as mybir
    from concourse import bass2jax

    nc = _get_nc()
    bass2jax.install_neuronx_cc_hook()

    partition_name = (nc.partition_id_tensor.name
                      if nc.partition_id_tensor else None)
    in_names, out_names, out_avals, zero_outs = [], [], [], []
    for alloc in nc.m.functions[0].allocations:
        if not isinstance(alloc, mybir.MemoryLocationSet):
            continue
        name = alloc.memorylocations[0].name
        if alloc.kind == "ExternalInput":
            if name != partition_name:
                in_names.append(name)
        elif alloc.kind == "ExternalOutput":
            out_names.append(name)
            shape = tuple(alloc.tensor_shape)
            dtype = mybir.dt.np(alloc.dtype)
            out_avals.append(jax.core.ShapedArray(shape, dtype))
            zero_outs.append(np.zeros(shape, dtype))
    n_params = len(in_names)
    all_names = in_names + out_names
    if partition_name is not None:
        all_names = all_names + [partition_name]

    def _body(*args):
        operands = list(args)
        if partition_name is not None:
            operands.append(bass2jax.partition_id_tensor())
        outs = bass2jax._bass_exec_p.bind(
            *operands,
            out_avals=tuple(out_avals),
            in_names=tuple(all_names),
            out_names=tuple(out_names),
            lowering_input_output_aliases=(),
            sim_require_finite=True,
            sim_require_nnan=True,
            nc=nc,
        )
        return tuple(outs)

    devices = jax.devices()[:N_CORES]
    mesh = Mesh(np.asarray(devices), ("core",))
    n_outs = len(out_names)
    sharded = jax.jit(
        shard_map(_body, mesh=mesh,
                  in_specs=(PartitionSpec("core"),) * (n_params + n_outs),
                  out_specs=(PartitionSpec("core"),) * n_outs,
                  check_rep=False),
        donate_argnums=tuple(range(n_params, n_params + n_outs)),
        keep_unused=True,
    )
    runner = {
        "fn": sharded, "mesh": mesh, "in_names": in_names,
        "out_names": out_names, "zero_outs": zero_outs, "n_params": n_params,
    }
    _CACHE["runner"] = runner
    return runner


def _shard_inputs(tri_feat_org, alpha, mask):
    """Concatenated per-core global inputs keyed by dram tensor name."""
    return {
        "feat": np.ascontiguousarray(tri_feat_org, dtype=np.float32),
        "alpha": np.ascontiguousarray(alpha, dtype=np.float32),
        "mask": np.ascontiguousarray(mask, dtype=np.float32)[..., None],
    }


def _combine(partials_global):
    """partials_global: (8, 4) array of per-core partial scalars."""
    pg = np.asarray(partials_global, dtype=np.float64).reshape(N_CORES, 4)
    sp, rg, nt = pg[:, 0].sum(), pg[:, 1].sum(), pg[:, 2].sum()
    denom = float(B) * S * R * D
    return np.float32(sp / nt + 0.01 * rg / denom)


def kernel(tri_feat_org, alpha, mask):
    r = _get_runner()
    named = _shard_inputs(tri_feat_org, alpha, mask)
    args = [named[n] for n in r["in_names"]]
    zeros = [np.zeros((N_CORES * z.shape[0], *z.shape[1:]), z.dtype)
             for z in r["zero_outs"]]
    outs = r["fn"](*args, *zeros)
    part = np.asarray(outs[r["out_names"].index("partials")])
    return np.asarray(_combine(part), dtype=np.float32)
